# revision 20
# baseline (speedup 1.0000x reference)
"""CISS-VAE (per-cluster MoE-routed MLP chain) Trainium2 kernel.

Strategy (routing on host, compute on device):
  - Rows are grouped by cluster label on the host; core c processes all rows
    of cluster c (C == n_cores == 8), so every GEMM is a dense per-cluster
    GEMM (no 8x redundant einsum like the reference).
  - The encoder (enc0, encu, enc2, fused mu|lv head) runs in fp8-e4m3 with
    DoubleRow matmuls (2 fp8 k-rows per cell, ~2x bf16 rate): the VAE latent
    z is dominated by the eps noise term, so encoder-side quantization error
    is attenuated far below the decoder's sensitivity (measured end-to-end
    rel err 1.9e-3 vs 1.8e-3 all-bf16). The decoder stays bf16 (decoder-side
    fp8 measured 1.2e-2..3.3e-2, too close to the 2e-2 gate).
  - fp8 scales chosen so every encoder PSUM eviction is scale-free
    (sigma_out == sigma_w * sigma_in): x*4, enc0 weights *16, hidden
    activations *64, encu/enc2 weights *1. Scale-free evictions are a
    2-ALU-op pattern (add bias, max 0) so they split across the Scalar AND
    Vector engines - eviction throughput, not the PE, limits the thin
    layers, so both engines share the drain work everywhere.
  - Schedule: enc0 runs as a plain layer-wavefront over the row blocks,
    then encu/enc2/mu|lv/latent/dec0/dec1/dec2/fin advance as one staggered
    software pipeline (layer L of block b interleaves with neighboring
    blocks' other layers), so thin stages always have thick matmuls in
    flight around them and the mu/sigma/z chain latency is hidden. npad is
    split into near-equal blocks (<=512 = one PSUM bank), largest first.
  - DMA descriptor issue costs ~0.65us of engine time each, so transfers
    are merged: one DMA per x block, per-k2 weight slabs, one merged
    [128, n_k*fo] tile per decoder layer, one bias blob, one output DMA per
    block. Issues spread across the sync/scalar/vector/gpsimd queues ahead
    of the eviction streams.
"""

import ml_dtypes
import numpy as np

import concourse.bacc as bacc
import concourse.mybir as mybir
import concourse.tile as tile
from concourse import bass_utils

P = 128
D_IN, LAT, C = 512, 64, 8
H0, H1, H2 = 1024, 512, 256
N_CORES = 8
F32 = mybir.dt.float32
BF16 = mybir.dt.bfloat16
F8 = mybir.dt.float8e4
AF = mybir.ActivationFunctionType
ALU = mybir.AluOpType
DR = mybir.MatmulPerfMode.DoubleRow
BF16_NP = ml_dtypes.bfloat16
F8_NP = ml_dtypes.float8_e4m3

S_X = 4.0    # fp8 scale on the x input
S_W0 = 16.0  # fp8 scale on enc0 weights
S_H = 64.0   # fp8 scale on encoder hidden activations (== S_X*S_W0 == 1*S_H)
S_WM = 16.0  # fp8 scale on the mu|lv head weights

ENC_LAYERS = dict(enc0=(D_IN, H0), encu=(H0, H1), enc2=(H1, H2), mulv=(H2, 2 * LAT), dec1=(H2, H1))
DEC_LAYERS = dict(dec0=(LAT, H2), dec2=(H1, H0), fin=(H0, D_IN))
BIAS_ORDER = ["enc0", "encu", "enc2", "mulv", "dec0", "dec1", "dec2", "fin"]


def _ceil_to(x, m):
    return ((x + m - 1) // m) * m


def _bias_cols(name):
    table = ENC_LAYERS if name in ENC_LAYERS else DEC_LAYERS
    return max(1, table[name][1] // P)


def _w8(w, scale):
    """[fi, fo] weight -> fp8 DR layout [128, n_k2, 2, fo]."""
    w = np.asarray(w, np.float32)
    fi, fo = w.shape
    n_k2 = max(1, fi // 256)
    a = (w * scale).astype(F8_NP).reshape(n_k2, 2, P, fo)
    return np.ascontiguousarray(a.transpose(2, 0, 1, 3))


def _wdec(w):
    """[fi, fo] bf16 weight -> merged [kp, n_k*fo] (k-slabs side by side)."""
    w = np.asarray(w, np.float32).astype(BF16_NP)
    fi, fo = w.shape
    if fi <= P:
        return np.ascontiguousarray(w)
    n_k = fi // P
    return np.ascontiguousarray(w.reshape(n_k, P, fo).transpose(1, 0, 2).reshape(P, n_k * fo))


def _blocks_of(npad):
    """Split npad into <=512 near-equal multiples of 64, descending."""
    n_blk = (npad + 511) // 512
    base = (npad // n_blk) // 64 * 64
    blocks = [base] * n_blk
    rem = npad - base * n_blk
    i = 0
    while rem > 0:
        blocks[i] += 64
        rem -= 64
        i = (i + 1) % n_blk
    return sorted(blocks, reverse=True)


def _build_module(npad, blocks):
    nc = bacc.Bacc("TRN2", target_bir_lowering=False, debug=False)
    n_blk = len(blocks)
    offs = [sum(blocks[:i]) for i in range(n_blk)]
    bmax = max(blocks)

    dram = {}

    def din(name, shape, dt):
        dram[name] = nc.dram_tensor(name, list(shape), dt, kind="ExternalInput").ap()
        return dram[name]

    din("x_dr", (P, 4, npad), F8)  # dim1 = k2*2 + grp
    din("epsT", (LAT, npad), F32)
    for name, (fi, fo) in ENC_LAYERS.items():
        din("w_" + name, (P, max(1, fi // 256), 2, fo), F8)
    for name, (fi, fo) in DEC_LAYERS.items():
        kp = min(P, fi)
        din("w_" + name, (kp, max(1, fi // P) * fo), BF16)
    n_bias = sum(_bias_cols(n) for n in BIAS_ORDER)
    din("biases", (P, n_bias), F32)
    outT = nc.dram_tensor("outT", [P, 4, npad], F32, kind="ExternalOutput").ap()

    with tile.TileContext(nc) as tc:
        with (
            tc.tile_pool(name="wpool", bufs=1) as wpool,
            tc.tile_pool(name="acts", bufs=2) as acts,
            tc.tile_pool(name="psum", bufs=8, space="PSUM") as psum,
        ):
            wsb = {}

            # ---- warm up the PE while engines/queues boot ----
            wu_w = wpool.tile([P, P], BF16, tag="wu_w", name="wu_w")
            wu_x = wpool.tile([P, P], BF16, tag="wu_x", name="wu_x")
            nc.vector.memset(wu_w[:], 0.0)
            nc.vector.memset(wu_x[:], 0.0)
            for _ in range(80):
                wu_ps = psum.tile([P, 512], F32, tag="ps", name="wu_ps")
                nc.tensor.matmul(wu_ps[:, :P], wu_w[:], wu_x[:], start=True, stop=True)

            # ---- all weight/input DMAs, merged + spread across queues ----
            x_in = [None] * n_blk

            def load_x(b, eng):
                nb, off = blocks[b], offs[b]
                t = acts.tile([P, 4, bmax], F8, tag="x", bufs=n_blk, name=f"x_{b}")
                eng.dma_start(t[:, :, :nb], dram["x_dr"][:, :, off : off + nb])
                x_in[b] = t

            def load_enc_w(name, k2s, eng):
                fi, fo = ENC_LAYERS[name]
                n_k2 = max(1, fi // 256)
                if name not in wsb:
                    wsb[name] = wpool.tile([P, n_k2, 2, fo], F8, tag=f"w_{name}", name=f"w_{name}")
                for k2 in k2s:
                    eng.dma_start(wsb[name][:, k2, :, :], dram["w_" + name][:, k2, :, :])

            def load_dec_w(name, eng):
                fi, fo = DEC_LAYERS[name]
                kp = min(P, fi)
                n_k = max(1, fi // P)
                t = wpool.tile([kp, n_k * fo], BF16, tag=f"w_{name}", name=f"w_{name}")
                eng.dma_start(t[:], dram["w_" + name][:])
                wsb[name] = t

            # prologue: first-MM deps first. x rides ahead of the decoder
            # flood on the sync HWDGE queue (in-order per queue); enc0
            # weights + bias blob on scalar (before its eviction stream);
            # the other encoder weights on the gpsimd SWDGE engine.
            # critical path (x(b0) + enc0 weights + bias) spread across all
            # three DMA-capable queues; per-queue DMA bandwidth is ~the
            # aggregate divided by active queues, so nothing critical may
            # sit behind a big transfer on one queue.
            bias_t = wpool.tile([P, sum(_bias_cols(n) for n in BIAS_ORDER)], F32, tag="biases", name="biases")
            nc.scalar.dma_start(bias_t[:], dram["biases"][:])
            # x(b0) halves + enc0 weight slab pieces spread over all queues
            nb0 = blocks[0]
            x0t = acts.tile([P, 4, bmax], F8, tag="x", bufs=n_blk, name="x_0")
            x_in[0] = x0t
            w0 = wpool.tile([P, 2, 2, H0], F8, tag="w_enc0", name="w_enc0")
            wsb["enc0"] = w0

            def wp(j, eng):
                eng.dma_start(
                    w0[:, :, :, 256 * j : 256 * (j + 1)],
                    dram["w_enc0"][:, :, :, 256 * j : 256 * (j + 1)],
                )

            nc.sync.dma_start(x0t[:, 0:2, :nb0], dram["x_dr"][:, 0:2, :nb0])
            wp(0, nc.scalar)
            wp(2, nc.gpsimd)
            nc.sync.dma_start(x0t[:, 2:4, :nb0], dram["x_dr"][:, 2:4, :nb0])
            wp(1, nc.sync)
            wp(3, nc.scalar)
            b_off = {}
            o = 0
            for n in BIAS_ORDER:
                b_off[n] = o
                o += _bias_cols(n)
            load_x(1, nc.sync)
            load_enc_w("encu", [0, 1, 2, 3], nc.gpsimd)
            for b in range(2, n_blk):
                load_x(b, nc.sync)
            load_enc_w("enc2", [0, 1], nc.gpsimd)
            load_enc_w("mulv", [0], nc.gpsimd)
            load_dec_w("dec0", nc.gpsimd)
            eps_t = acts.tile([LAT, npad], F32, tag="eps", bufs=1, name="eps")
            nc.sync.dma_start(eps_t[:], dram["epsT"][:])
            load_enc_w("dec1", [0], nc.sync)
            for name in ("dec2", "fin"):
                load_dec_w(name, nc.sync)

            def bias_ap(name, m, p0=0, p1=P):
                return bias_t[p0:p1, b_off[name] + m : b_off[name] + m + 1]

            # ---- activation tiles ----
            h0 = [[None] * 4 for _ in range(n_blk)]
            h1 = [[None] * 2 for _ in range(n_blk)]
            h2 = [None] * n_blk
            mu_t = [None] * n_blk
            sg_t = [None] * n_blk
            z_t = [None] * n_blk
            h3 = [None] * n_blk
            h4 = [[None] * 4 for _ in range(n_blk)]
            out_t = [None] * n_blk
            h5 = [[None] * 8 for _ in range(n_blk)]

            def mm_fp8(name, b, ins_of, evict):
                nb = blocks[b]
                fi, fo = ENC_LAYERS[name]
                w_t = wsb[name]
                n_k2 = max(1, fi // 256)
                n_m = max(1, fo // P)
                mp = min(P, fo)
                for m in range(n_m):
                    ps = psum.tile([P, 512], F32, tag="ps", name=f"ps_{name}_{m}_{b}")
                    for k2 in range(n_k2):
                        nc.tensor.matmul(
                            ps[:mp, :nb],
                            w_t[:, k2, :, m * mp : (m + 1) * mp],
                            ins_of(k2),
                            start=(k2 == 0),
                            stop=(k2 == n_k2 - 1),
                            perf_mode=DR,
                        )
                    evict(m, ps)

            def mm_bf16(name, b, ins, evict):
                nb = blocks[b]
                fi, fo = DEC_LAYERS[name]
                w_t = wsb[name]
                n_k = max(1, fi // P)
                n_m = max(1, fo // P)
                mp = min(P, fo)
                for m in range(n_m):
                    ps = psum.tile([P, 512], F32, tag="ps", name=f"ps_{name}_{m}_{b}")
                    for k in range(n_k):
                        nc.tensor.matmul(
                            ps[:mp, :nb],
                            w_t[:, k * fo + m * mp : k * fo + (m + 1) * mp],
                            ins[k][:, :nb],
                            start=(k == 0),
                            stop=(k == n_k - 1),
                        )
                    evict(m, ps)

            def relu_evict(eng, out_ap, ps_ap, b_ap):
                if eng is nc.scalar:
                    nc.scalar.activation(out_ap, ps_ap, AF.Relu, bias=b_ap, scale=1.0)
                else:
                    eng.tensor_scalar(out_ap, ps_ap, b_ap, 0.0, ALU.add, ALU.max)

            # ---- per-layer stages ----
            def enc0_stage(b):
                nb = blocks[b]
                xt = x_in[b]

                def ev(m, ps):
                    t = h0[b][m // 2]
                    if t is None:
                        t = acts.tile([P, 2, bmax], F8, tag=f"h0_{m // 2}", bufs=n_blk, name=f"h0_{m // 2}_{b}")
                        h0[b][m // 2] = t
                    eng = nc.scalar if m % 2 == 0 else nc.vector
                    relu_evict(eng, t[:, m % 2, :nb], ps[:, :nb], bias_ap("enc0", m))

                mm_fp8("enc0", b, lambda k2: xt[:, 2 * k2 : 2 * k2 + 2, :nb], ev)

            def encu_stage(b):
                nb = blocks[b]

                def ev(m, ps):
                    t = h1[b][m // 2]
                    if t is None:
                        t = acts.tile([P, 2, bmax], F8, tag=f"h1_{m // 2}", bufs=n_blk, name=f"h1_{m // 2}_{b}")
                        h1[b][m // 2] = t
                    eng = nc.scalar if m % 2 == 0 else nc.vector
                    relu_evict(eng, t[:, m % 2, :nb], ps[:, :nb], bias_ap("encu", m))

                mm_fp8("encu", b, lambda k2: h0[b][k2][:, :, :nb], ev)

            def enc2_stage(b):
                nb = blocks[b]

                def ev(m, ps):
                    t = h2[b]
                    if t is None:
                        t = acts.tile([P, 2, bmax], F8, tag="h2", bufs=n_blk, name=f"h2_{b}")
                        h2[b] = t
                    eng = nc.scalar if m % 2 == 0 else nc.vector
                    relu_evict(eng, t[:, m, :nb], ps[:, :nb], bias_ap("enc2", m))

                mm_fp8("enc2", b, lambda k2: h1[b][k2][:, :, :nb], ev)

            def mulv_stage(b):
                nb = blocks[b]

                def ev(m, ps):
                    mu = acts.tile([LAT, bmax], F32, tag="mu", bufs=2, name=f"mu_{b}")
                    sg = acts.tile([LAT, bmax], F32, tag="sg", bufs=2, name=f"sg_{b}")
                    nc.vector.tensor_scalar(
                        mu[:, :nb], ps[:LAT, :nb], 1.0 / (S_H * S_WM),
                        bias_ap("mulv", 0, 0, LAT), ALU.mult, ALU.add,
                    )
                    nc.scalar.activation(
                        sg[:, :nb], ps[LAT:, :nb], AF.Exp,
                        bias=bias_ap("mulv", 0, LAT, P), scale=0.5 / (S_H * S_WM),
                    )
                    mu_t[b], sg_t[b] = mu, sg

                mm_fp8("mulv", b, lambda k2: h2[b][:, :, :nb], ev)

            def lat_stage(b):
                nb, off = blocks[b], offs[b]
                tmp = acts.tile([LAT, bmax], F32, tag="tmp", bufs=2, name=f"tmp_{b}")
                nc.vector.tensor_mul(tmp[:, :nb], sg_t[b][:, :nb], eps_t[:, off : off + nb])
                z = acts.tile([LAT, bmax], BF16, tag="z", bufs=n_blk, name=f"z_{b}")
                nc.vector.tensor_add(z[:, :nb], tmp[:, :nb], mu_t[b][:, :nb])
                z_t[b] = z

            def dec0_stage(b):
                nb = blocks[b]

                def ev(m, ps):
                    t = h3[b]
                    if t is None:
                        t = acts.tile([P, 2, bmax], F8, tag="h3", bufs=n_blk, name=f"h3_{b}")
                        h3[b] = t
                    # h3 stored fp8 at scale 64 for the DR dec1 layer
                    nc.scalar.activation(
                        t[:, m, :nb], ps[:, :nb], AF.Relu,
                        bias=bias_ap("dec0", m), scale=64.0,
                    )

                mm_bf16("dec0", b, [z_t[b]], ev)

            def dec1_stage(b):
                nb = blocks[b]

                def ev(m, ps):
                    t = acts.tile([P, bmax], BF16, tag=f"h4_{m}", bufs=n_blk, name=f"h4_{m}_{b}")
                    nc.scalar.activation(
                        t[:, :nb], ps[:, :nb], AF.Relu,
                        bias=bias_ap("dec1", m), scale=1.0 / (64.0 * 32.0),
                    )
                    h4[b][m] = t

                mm_fp8("dec1", b, lambda k2: h3[b][:, :, :nb], ev)

            def dec2_stage(b):
                nb = blocks[b]

                def ev(m, ps):
                    t = acts.tile([P, bmax], BF16, tag=f"h5_{m}", bufs=n_blk, name=f"h5_{m}_{b}")
                    eng = nc.scalar if m % 2 == 0 else nc.vector
                    relu_evict(eng, t[:, :nb], ps[:, :nb], bias_ap("dec2", m))
                    h5[b][m] = t

                mm_bf16("dec2", b, h4[b], ev)

            def fin_stage(b):
                nb, off = blocks[b], offs[b]
                ot = acts.tile([P, 4, bmax], F32, tag="out", bufs=2, name=f"out_{b}")
                out_t[b] = ot

                def ev(m, ps):
                    if m % 2 == 0:
                        nc.scalar.activation(
                            ot[:, m, :nb], ps[:, :nb], AF.Identity,
                            bias=bias_ap("fin", m), scale=1.0,
                        )
                    else:
                        nc.vector.tensor_scalar(
                            ot[:, m, :nb], ps[:, :nb], bias_ap("fin", m), None, ALU.add
                        )
                    if b == n_blk - 1:
                        # tail block: split the store across two queues so the
                        # final transfer isn't serialized behind earlier outs
                        h = nb // 2
                        nc.scalar.dma_start(outT[:, m, off : off + h], ot[:, m, :h])
                        nc.sync.dma_start(outT[:, m, off + h : off + nb], ot[:, m, h:nb])
                    else:
                        nc.sync.dma_start(outT[:, m, off : off + nb], ot[:, m, :nb])

                mm_bf16("fin", b, h5[b], ev)

            # ---- schedule: one staggered software pipeline over all layers ----
            def maybe(stage, b):
                if 0 <= b < n_blk:
                    stage(b)

            # mu|lv + latent chain early in the iteration, dec0 (their
            # consumer) last: ~4-8us of other matmuls always sit between
            # z's production and its first use, hiding the chain latency.
            for i in range(n_blk + 8):
                maybe(enc0_stage, i)
                maybe(encu_stage, i - 2)
                maybe(enc2_stage, i - 3)
                maybe(mulv_stage, i - 4)
                maybe(lat_stage, i - 4)
                maybe(dec1_stage, i - 6)
                maybe(dec2_stage, i - 7)
                maybe(fin_stage, i - 8)
                maybe(dec0_stage, i - 5)

    nc.compile()
    return nc


def kernel(**inputs):
    x = np.asarray(inputs["x"], dtype=np.float32)
    lbl = np.asarray(inputs["cluster_labels"]).astype(np.int64)
    eps = np.asarray(inputs["eps"], dtype=np.float32)
    B = x.shape[0]

    counts = np.bincount(lbl, minlength=C)
    npad = max(512, _ceil_to(int(counts.max()), 64))
    blocks = _blocks_of(npad)

    rows = [np.nonzero(lbl == c)[0] for c in range(C)]

    mulv_W = np.concatenate([np.asarray(inputs["mu_W"]), np.asarray(inputs["lv_W"])], axis=1)
    mulv_b = np.concatenate([np.asarray(inputs["mu_b"]), 0.5 * np.asarray(inputs["lv_b"])])

    def bias_blob(per_cluster):
        cols = []
        for name in BIAS_ORDER:
            b = per_cluster[name]
            f = b.shape[0]
            if f >= P:
                cols.append(b.reshape(f // P, P).T)
            else:
                cols.append(np.tile(b.reshape(1, f).T, (P // f, 1)).reshape(P, 1))
        return np.ascontiguousarray(np.concatenate(cols, axis=1).astype(np.float32))

    shared_w = {
        "w_enc0": _w8(inputs["enc_W0"], S_W0),
        "w_enc2": _w8(inputs["enc_W2"], 1.0),
        "w_mulv": _w8(mulv_W, S_WM),
        "w_dec1": _w8(inputs["dec_W1"], 32.0),
    }

    in_maps = []
    for c in range(C):
        r = rows[c]
        xT = np.zeros((D_IN, npad), np.float32)
        xT[:, : len(r)] = x[r].T
        x_dr = (xT * S_X).astype(F8_NP).reshape(4, P, npad).transpose(1, 0, 2)
        epsT = np.zeros((LAT, npad), np.float32)
        epsT[:, : len(r)] = eps[r].T
        m = dict(shared_w)
        m["x_dr"] = np.ascontiguousarray(x_dr)
        m["epsT"] = epsT
        m["w_encu"] = _w8(inputs["enc_Wu"][c], 1.0)
        m["w_dec0"] = _wdec(inputs["dec_Wu0"][c])
        m["w_dec2"] = _wdec(inputs["dec_Wu2"][c])
        m["w_fin"] = _wdec(inputs["fin_W"][c])
        m["biases"] = bias_blob({
            "enc0": S_H * np.asarray(inputs["enc_b0"]),
            "encu": S_H * np.asarray(inputs["enc_bu"][c]),
            "enc2": S_H * np.asarray(inputs["enc_b2"]),
            "mulv": mulv_b,
            "dec0": 64.0 * np.asarray(inputs["dec_bu0"][c]),
            "dec1": np.asarray(inputs["dec_b1"]),
            "dec2": np.asarray(inputs["dec_bu2"][c]),
            "fin": np.asarray(inputs["fin_b"][c]),
        })
        in_maps.append(m)

    nc = _build_module(npad, blocks)
    res = bass_utils.run_bass_kernel_spmd(nc, in_maps, core_ids=list(range(N_CORES)))
    global LAST_RESULTS
    LAST_RESULTS = res

    out = np.empty((B, D_IN), np.float32)
    for c in range(C):
        r = rows[c]
        o = res.results[c]["outT"]  # [128, 4, npad]
        out[r] = o.transpose(1, 0, 2).reshape(D_IN, npad)[:, : len(r)].T
    return out
